# revision 11
# baseline (speedup 1.0000x reference)
"""DecomposedEmbedding lookup on 8 trn2 NeuronCores.

weight = sw * sigmoid(mask)[:,None] + aw + sum_k(atten[k] * from_kb[...,k]);
out = weight[input_ids].

Strategy (tensor parallel on the embedding table, per the vocab-sharding
hint): the host packs the four vocab tables into one combined bf16 table
comb[v] = [sw | aw | from_kb[...,0] | from_kb[...,1] | mask | pad] (640 bf16,
1280B rows; bf16 keeps rel-err ~2e-3, well inside the 2e-2 gate, and halves
the gather bytes vs f32) and shards it row-wise across the 8 cores (62500
rows each).  Tokens are bucketed by owning core and by 31250-row half-shard
(so shard-local indices fit the gather engine's int16 index type).  Each core
runs batched DMA-gathers (<=1024 indices each, a hardware limit) pulling only
the 1028B row prefix it needs, then combines on-chip: sigmoid + atten
scalings on ACT, per-token row scaling and adds on DVE.  The host scatters
each core's result rows back into token order (the inverse bucketing
permutation), which replaces the all-to-all since the full output is
assembled on host anyway.
"""

import os

import ml_dtypes
import numpy as np

BF16 = ml_dtypes.bfloat16

V = 500000
D = 128
K = 2
NCORES = 8
VS = V // NCORES  # rows per core
HALF = VS // 2  # rows per half-shard (int16-indexable)
P = 128
QB = 1024  # max indices per dma_gather instruction

# packed-row layouts per dtype: row = sw|aw|f0|f1|mask|pad, gathered prefix
# covers sw..mask. Row stride must be a multiple of 256B for dma_gather.
_LAYOUT = {
    "bf16": dict(E=4 * D + 128, EG=4 * D + 2),  # 1280B rows, 1028B gathered
    "f32": dict(E=4 * D + 64, EG=4 * D + 2),  # 2304B rows, 2056B gathered
}


def _dt():
    return os.environ.get("KDT", "bf16")


def _block_sizes(caph):
    """Split caph tokens into near-equal blocks of <=QB, multiples of 128."""
    nblk = -(-caph // QB)
    g = caph // P
    gper, rem = divmod(g, nblk)
    return [(gper + (1 if i < rem else 0)) * P for i in range(nblk)]


LAST_EXEC_TIME_NS = None
LAST_RESULTS = None

_PROG_CACHE = {}


def _build_program(caph):
    from concourse import bacc, tile
    import concourse.mybir as mybir

    kdt = _dt()
    lay = _LAYOUT[kdt]
    E, EG = lay["E"], lay["EG"]
    dt = mybir.dt.float32 if kdt == "f32" else mybir.dt.bfloat16
    dtsz = 4 if kdt == "f32" else 2

    kloop = int(os.environ.get("KLOOP", "0"))
    no_gather = os.environ.get("KNO_GATHER", "") == "1"
    no_compute = os.environ.get("KNO_COMPUTE", "") == "1"
    no_out = os.environ.get("KNO_OUT", "") == "1"
    bufs = int(os.environ.get("KBUFS", "3"))
    eg = int(os.environ.get("KELEM", str(EG)))
    single_pkt = os.environ.get("KSP", "1") == "1"
    nqueues = int(os.environ.get("KQN", "1"))

    f32, i16 = mybir.dt.float32, mybir.dt.int16
    nc = bacc.Bacc(
        "TRN2",
        target_bir_lowering=False,
        debug=False,
        enable_asserts=False,
        num_devices=NCORES,
        num_swdge_queues=nqueues,
    )
    comb = nc.dram_tensor("comb", [VS, E], dt, kind="ExternalInput")
    attn = nc.dram_tensor("attn", [P, K], f32, kind="ExternalInput")
    S = caph // 16
    idx = nc.dram_tensor("idx", [2, P, S], i16, kind="ExternalInput")
    out = nc.dram_tensor("out", [2 * caph, D], dt, kind="ExternalOutput")

    mult = mybir.AluOpType.mult

    def compute(gt, rt, Gb):
        # tensor_scalar requires an f32 scalar operand
        sig = wpool.tile([P, Gb], f32, tag="sig")
        nc.scalar.activation(
            out=sig[:],
            in_=gt[:, 0:Gb, 4 * D],
            func=mybir.ActivationFunctionType.Sigmoid,
        )
        # rt[:, g, :] = sw_row * sigmoid(mask_row): per-partition scalar
        # multiply; per-group 2x-mode tensor_scalar is cheap on DVE
        for g in range(Gb):
            nc.vector.tensor_scalar(
                out=rt[:, g, :], in0=gt[:, g, 0:D],
                scalar1=sig[:, g : g + 1], scalar2=None, op0=mult,
            )
        u = wpool.tile([P, Gb, D], dt, tag="u")
        v = wpool.tile([P, Gb, D], dt, tag="v")
        # atten scalings on ACT (runs parallel to DVE)
        nc.scalar.mul(out=u[:], in_=gt[:, 0:Gb, 2 * D : 3 * D], mul=attn_t[:, 0:1])
        nc.scalar.mul(out=v[:], in_=gt[:, 0:Gb, 3 * D : 4 * D], mul=attn_t[:, 1:2])
        nc.vector.tensor_add(out=rt[:], in0=rt[:], in1=gt[:, 0:Gb, D : 2 * D])
        nc.vector.tensor_add(out=rt[:], in0=rt[:], in1=u[:])
        nc.vector.tensor_add(out=rt[:], in0=rt[:], in1=v[:])

    def raw_gather(out_ap, in_ap, idxs_ap, num_idxs, elem_size, elem_step, qn=0):
        # bass.dma_gather minus the elem_size%256 assert (a transpose-path
        # restriction); row stride must still be a multiple of 256B.
        g = nc.gpsimd
        stride_bytes = elem_step * dtsz
        assert stride_bytes % 256 == 0
        stride_bytes_256 = stride_bytes // 256
        _in_ap = g.lower_ap_dma(in_ap, for_custom_bir_dma=True)
        _idxs_ap = g.lower_ap(idxs_ap)
        _out_ap = g.lower_ap(out_ap)
        return g.add_instruction(
            mybir.InstDMAGatherAnt(
                name=g.bass.get_next_instruction_name(),
                ins=[*_in_ap, _idxs_ap, g.lower_val_access(g.to_reg(num_idxs))],
                outs=[_out_ap],
                transpose=False,
                num_idxs=num_idxs,
                elem_size=elem_size,
                stride_bytes_256=stride_bytes_256,
                gen_mode=0,
                single_packet=single_pkt,
                queue_num=qn,
                sbuf_tokens_per_rank=0,
                sbuf_free_dim_per_rank=0,
                sbuf_free_dim_pad_per_rank=0,
                sbuf_byte_offset=0,
            )
        )

    def body():
        # pipeline unit: one <=QB-token gather block (HW limit per dma_gather)
        gi = 0
        for h in range(2):
            idx_t = wpool.tile([P, S], i16, tag="idx")
            nc.sync.dma_start(out=idx_t[:], in_=idx[h])
            b0 = 0
            for nb in _block_sizes(caph):
                Gb = nb // P
                if no_gather:
                    gt = gtc  # pre-filled const tile (ablation only)
                else:
                    gt = wpool.tile([P, Gb, eg], dt, tag="gt")
                if not no_gather:
                    if eg == E:
                        nc.gpsimd.dma_gather(
                            out_ap=gt[:],
                            in_ap=comb[h * HALF : (h + 1) * HALF, :],
                            idxs_ap=idx_t[:, b0 // 16 : (b0 + nb) // 16],
                            num_idxs=nb,
                            num_idxs_reg=nb,
                            elem_size=E,
                            single_packet=single_pkt,
                            queue_num=gi % nqueues,
                        )
                    else:
                        raw_gather(
                            out_ap=gt[:],
                            in_ap=comb[h * HALF : (h + 1) * HALF, 0:eg],
                            idxs_ap=idx_t[:, b0 // 16 : (b0 + nb) // 16],
                            num_idxs=nb,
                            elem_size=eg,
                            elem_step=E,
                            qn=gi % nqueues,
                        )
                gi += 1
                rt = wpool.tile([P, Gb, D], dt, tag="rt")
                if no_compute:
                    nc.vector.tensor_copy(out=rt[:], in_=gt[:, 0:Gb, 0:D])
                else:
                    compute(gt, rt, Gb)
                if not no_out:
                    nc.sync.dma_start(
                        out=out[h * caph + b0 : h * caph + b0 + nb, :].rearrange(
                            "(g p) d -> p g d", p=P
                        ),
                        in_=rt[:],
                    )
                b0 += nb

    with tile.TileContext(nc) as tc:
        with (
            tc.tile_pool(name="const", bufs=1) as cpool,
            tc.tile_pool(name="work", bufs=bufs) as wpool,
        ):
            attn_t = cpool.tile([P, K], f32)
            nc.sync.dma_start(out=attn_t[:], in_=attn[:])
            gtc = None
            if no_gather:
                gmax = max(_block_sizes(caph)) // P
                gtc = cpool.tile([P, gmax, eg], dt, tag="gtc")
                nc.vector.memset(gtc[:], 0.5)

            if kloop:
                with tc.For_i(0, kloop, 1):
                    body()
            else:
                body()

    nc.compile()
    return nc


def _pack_idx(loc, caph):
    """int16 local row ids -> [P, caph//16], wrapped-by-16 per QB-token
    gather block, replicated across the 8 groups of 16 partitions."""
    arr = np.zeros(caph, dtype=np.int16)
    arr[: len(loc)] = loc
    cols = []
    b0 = 0
    for nb in _block_sizes(caph):
        blk = arr[b0 : b0 + nb]
        cols.append(blk.reshape(nb // 16, 16).T)
        b0 += nb
    return np.tile(np.concatenate(cols, axis=1), (8, 1))


def _host_pack(input_ids, sw, mask, aw, atten, from_kb):
    """Bucket tokens by (core, half-shard) and pack the combined table.

    Returns (in_maps, sels, caph, n_tok, ids_shape)."""
    kdt = _dt()
    lay = _LAYOUT[kdt]
    E = lay["E"]
    np_dt = np.float32 if kdt == "f32" else BF16

    ids_in = np.asarray(input_ids)
    ids = ids_in.reshape(-1)
    n_tok = ids.shape[0]
    sw = np.asarray(sw, dtype=np.float32)
    aw = np.asarray(aw, dtype=np.float32)
    mask = np.asarray(mask, dtype=np.float32)
    atten = np.asarray(atten, dtype=np.float32)
    from_kb = np.asarray(from_kb, dtype=np.float32)

    comb = np.zeros((V, E), dtype=np_dt)
    comb[:, 0:D] = sw.astype(np_dt)
    comb[:, D : 2 * D] = aw.astype(np_dt)
    comb[:, 2 * D : 3 * D] = from_kb[:, :, 0].astype(np_dt)
    comb[:, 3 * D : 4 * D] = from_kb[:, :, 1].astype(np_dt)
    comb[:, 4 * D] = mask.astype(np_dt)
    attn_r = np.ascontiguousarray(np.broadcast_to(atten[None, :], (P, K)))

    core_of = ids // VS
    rem = ids - core_of * VS
    half_of = rem // HALF
    loc = (rem - half_of * HALF).astype(np.int16)

    sels = [
        [np.flatnonzero((core_of == c) & (half_of == h)) for h in range(2)]
        for c in range(NCORES)
    ]
    maxb = max(max(len(s) for s in hs) for hs in sels)
    caph = max(((maxb + P - 1) // P) * P, P)

    in_maps = []
    for c in range(NCORES):
        idx_arr = np.stack(
            [_pack_idx(loc[sels[c][h]], caph) for h in range(2)], axis=0
        )
        in_maps.append(
            {
                "comb": comb[c * VS : (c + 1) * VS],
                "attn": attn_r,
                "idx": idx_arr,
            }
        )
    return in_maps, sels, caph, n_tok, ids_in.shape


def kernel(input_ids, sw, mask, aw, atten, from_kb):
    global LAST_EXEC_TIME_NS, LAST_RESULTS
    from concourse.bass_utils import run_bass_kernel_spmd

    in_maps, sels, caph, n_tok, ids_shape = _host_pack(
        input_ids, sw, mask, aw, atten, from_kb
    )

    key = (caph, _dt())
    if key not in _PROG_CACHE:
        _PROG_CACHE[key] = _build_program(caph)
    nc = _PROG_CACHE[key]

    res = run_bass_kernel_spmd(nc, in_maps, core_ids=list(range(NCORES)))
    LAST_EXEC_TIME_NS = getattr(res, "exec_time_ns", None)
    LAST_RESULTS = res

    full = np.empty((n_tok, D), dtype=np.float32)
    for c in range(NCORES):
        o = np.asarray(res.results[c]["out"], dtype=np.float32)
        for h in range(2):
            sel = sels[c][h]
            if len(sel):
                full[sel] = o[h * caph : h * caph + len(sel)]
    return full.reshape(*ids_shape, D)
